# revision 22
# baseline (speedup 1.0000x reference)
"""Bilinear RoI pooling (7x7) on 8 Trainium2 NeuronCores.

Strategy (data-parallel over RoIs, per the sharding hint):
  - B=1024 boxes split into 8 slices of 128; the feature map is replicated.
  - Host builds a pair-interleaved fp16 copy of the zero-padded feature map:
    padded map P2 is (132,132,C) with a 2-px zero border; two row-pair copies
    E (rows 2e,2e+1) and O (rows 2o+1,2o+2) are stored as [pair, x, r, c] so
    the full 2x2 bilinear patch for any (y0,x0) corner is 4*C CONTIGUOUS fp16
    elements -> ONE 4KB gather descriptor per (box, grid point).
  - Host also precomputes the per-(box,point) gather slot index and the four
    bilinear corner weights (O(B*49) scalar work, same spirit as the host-side
    feature padding; the O(B*49*C) gather+blend stays on device).
  - Per core, per (box, grid-point): one indirect-DMA gather of 4*C fp16,
    then a 4-term weighted sum split across the vector and scalar engines
    (2 muls on ACT, 2 muls + 3 adds on DVE) so neither engine exceeds the
    DMA roofline.
  - Out-of-bounds corners read zero border rows/cols (clamped indices), so no
    in-bounds masking is needed.

Device layout: partition = box (128/core); 49 grid points along free dim.
Output is fp16 on device; host casts to f32.
"""

import numpy as np

P = 128          # boxes per core == SBUF partitions
C = 512          # channels
NPT = 49         # 7*7 grid points
WP2 = 132        # padded width (2 zero cols each side)
HP2 = 132        # padded height (2 zero rows top, 2 bottom)
NBLK_E = 66      # even row-pairs (rows 0..131)
NBLK_O = 65      # odd row-pairs (rows 1..130)
NSLOT_E = NBLK_E * WP2
NSLOT = (NBLK_E + NBLK_O) * WP2   # 17292 slots of [2, C]
NCORES = 8

_STATE = {}


# NOTE: multi-index offset APs (merge_ab / gk>1 style) pass CoreSim but
# produce garbage on real hardware — the HW indirect DMA only honors [P,1]
# offsets. One gather per (box, grid point); the pair-interleaved layout
# makes that one gather cover all 4 bilinear corners.
def _build_nc(repeats=1, bufs=16, abufs=4, tbufs=6, store_group=7,
              split_tail=True, mode="actsplit"):
    """mode: 'actsplit' (2 muls on ACT engine) | 'full' (all-DVE blend) |
    'noblend' (gather+copy+store) | 'nodma' (blend from const tile)."""
    import concourse.bass as bass
    import concourse.bacc as bacc
    import concourse.tile as tile
    from concourse import mybir

    F32 = mybir.dt.float32
    F16 = mybir.dt.float16
    I32 = mybir.dt.int32
    Alu = mybir.AluOpType

    nc = bacc.Bacc()
    fgat = nc.declare_dram_parameter("fgat", [NSLOT, 2 * C], F16, isOutput=False)
    # meta = [idx-as-i32 | w00 | w10 | w01 | w11]; idx loaded first (it gates
    # the first gather), weights in parallel on the second HWDGE ring
    meta = nc.declare_dram_parameter("meta", [P, 5 * NPT], F32, isOutput=False)
    out = nc.declare_dram_parameter("out", [P, NPT * C], F16, isOutput=True)

    with tile.TileContext(nc) as tc:
        with (
            tc.tile_pool(name="const", bufs=1) as cpool,
            tc.tile_pool(name="apool", bufs=abufs) as apool,
            tc.tile_pool(name="gpool", bufs=bufs) as wpool,
            tc.tile_pool(name="tpool", bufs=tbufs) as tpool,
        ):
            # idx is int32 BITS stored in the f32 meta tensor: bitcast view,
            # no on-device cast needed
            idx = cpool.tile([P, NPT], I32)
            nc.sync.dma_start(out=idx[:], in_=meta[:, 0:NPT].bitcast(I32))
            w = cpool.tile([P, 4 * NPT], F32)
            nc.scalar.dma_start(out=w[:], in_=meta[:, NPT:5 * NPT])
            # gathered layout per point: [A0, B0, A1, B1] = [(y0,x0),(y0+1,x0),
            # (y0,x0+1),(y0+1,x0+1)]
            w00 = w[:, 0 * NPT:1 * NPT]
            w10 = w[:, 1 * NPT:2 * NPT]
            w01 = w[:, 2 * NPT:3 * NPT]
            w11 = w[:, 3 * NPT:4 * NPT]

            import concourse.bass as _b

            gconst = None
            if mode == "nodma":
                gconst = cpool.tile([P, 4 * C], F16, tag="gconst")
                nc.vector.memset(gconst[:], 0.25)

            # store plan: big groups first, small ones last so the final
            # HBM-write drain (the kernel tail) is short
            sg = store_group
            assert NPT % sg == 0
            plan = ([sg] * (NPT // sg - 1) + [4, 2, 1]) if split_tail \
                else [sg] * (NPT // sg)
            assert sum(plan) == NPT
            bases = [sum(plan[:i]) for i in range(len(plan))]
            for rep in range(repeats):
                for g_i, gn in enumerate(plan):
                    afat = apool.tile([P, sg * C], F16, tag="afat")
                    for k in range(gn):
                        t = bases[g_i] + k
                        if mode == "nodma":
                            g4 = gconst
                        else:
                            g4 = wpool.tile([P, 4 * C], F16, tag="g4")
                            nc.gpsimd.indirect_dma_start(
                                out=g4[:], out_offset=None, in_=fgat[:],
                                in_offset=_b.IndirectOffsetOnAxis(
                                    ap=idx[:, t:t + 1], axis=0))
                        ac = afat[:, k * C:(k + 1) * C]
                        if mode == "noblend":
                            nc.vector.tensor_copy(out=ac, in_=g4[:, 0:C])
                            continue
                        if mode in ("actsplit", "nodma"):
                            u1 = tpool.tile([P, C], F16, tag="u1")
                            nc.scalar.mul(u1[:], g4[:, C:2 * C], w10[:, t:t + 1])
                            u2 = tpool.tile([P, C], F16, tag="u2")
                            nc.scalar.mul(u2[:], g4[:, 3 * C:4 * C], w11[:, t:t + 1])
                            t1 = tpool.tile([P, C], F16, tag="t1")
                            nc.vector.tensor_scalar(
                                out=t1[:], in0=g4[:, 0:C], scalar1=w00[:, t:t + 1],
                                scalar2=None, op0=Alu.mult)
                            t2 = tpool.tile([P, C], F16, tag="t2")
                            nc.vector.tensor_scalar(
                                out=t2[:], in0=g4[:, 2 * C:3 * C],
                                scalar1=w01[:, t:t + 1],
                                scalar2=None, op0=Alu.mult)
                            nc.vector.tensor_tensor(out=t1[:], in0=t1[:],
                                                    in1=u1[:], op=Alu.add)
                            nc.vector.tensor_tensor(out=t2[:], in0=t2[:],
                                                    in1=u2[:], op=Alu.add)
                            nc.vector.tensor_tensor(out=ac, in0=t1[:],
                                                    in1=t2[:], op=Alu.add)
                            continue
                        nc.vector.tensor_scalar(
                            out=ac, in0=g4[:, 0:C], scalar1=w00[:, t:t + 1],
                            scalar2=None, op0=Alu.mult)
                        nc.vector.scalar_tensor_tensor(
                            out=ac, in0=g4[:, C:2 * C], scalar=w10[:, t:t + 1],
                            in1=ac, op0=Alu.mult, op1=Alu.add)
                        nc.vector.scalar_tensor_tensor(
                            out=ac, in0=g4[:, 2 * C:3 * C], scalar=w01[:, t:t + 1],
                            in1=ac, op0=Alu.mult, op1=Alu.add)
                        nc.vector.scalar_tensor_tensor(
                            out=ac, in0=g4[:, 3 * C:4 * C], scalar=w11[:, t:t + 1],
                            in1=ac, op0=Alu.mult, op1=Alu.add)
                    nc.sync.dma_start(
                        out=out[:, bases[g_i] * C:(bases[g_i] + gn) * C],
                        in_=afat[:, 0:gn * C])

    nc.compile()
    return nc


def _prep_fgat(features):
    """Pair-interleaved fp16 gather map: E (even row pairs) then O (odd)."""
    f = np.asarray(features, dtype=np.float32)
    p2 = np.zeros((HP2, WP2, C), dtype=np.float16)
    p2[2:130, 2:130, :] = f.astype(np.float16)
    # E[e, x, r, c] = p2[2e+r, x, c]; O[o, x, r, c] = p2[2o+1+r, x, c]
    e = np.ascontiguousarray(
        p2.reshape(NBLK_E, 2, WP2, C).transpose(0, 2, 1, 3)
    ).reshape(NSLOT_E, 2 * C)
    o = np.ascontiguousarray(
        p2[1:131].reshape(NBLK_O, 2, WP2, C).transpose(0, 2, 1, 3)
    ).reshape(NBLK_O * WP2, 2 * C)
    return np.concatenate([e, o], axis=0)


def _prep_wts_idx(boxes):
    """Per-(box,point) gather slot index and bilinear corner weights.

    Mirrors the reference affine-grid math in float32:
      yf = BY*(0.5*bh-0.5) + (yc-1),  xf = BX*(0.5*bw-0.5) + (xc-1)
    with BY/BX the 7x7 [-1,1] grid; then y0=floor(yf), wy=yf-y0 (same for x).
    OOB corners are mapped to zero border rows/cols of the padded map, so the
    weights need no in-bounds masking.
    """
    b = np.asarray(boxes, dtype=np.float32)
    xc, yc, bw, bh = b[:, 0:1], b[:, 1:2], b[:, 2:3], b[:, 3:4]
    base = np.linspace(-1.0, 1.0, 7).astype(np.float32)
    BY = np.repeat(base, 7)[None, :]   # (1,49)
    BX = np.tile(base, 7)[None, :]
    yf = (BY * (np.float32(0.5) * bh - np.float32(0.5)) + (yc - 1)).astype(np.float32)
    xf = (BX * (np.float32(0.5) * bw - np.float32(0.5)) + (xc - 1)).astype(np.float32)
    y0 = np.floor(yf)
    x0 = np.floor(xf)
    wy = yf - y0
    wx = xf - x0
    wyc = np.float32(1.0) - wy
    wxc = np.float32(1.0) - wx
    # weights for gathered layout [A0, B0, A1, B1]
    wts = np.concatenate([wyc * wxc, wy * wxc, wyc * wx, wy * wx], axis=1)
    # slot = par*NSLOT_E + half*WP2 + clamp(x0,-2,128)+2
    pyA = np.clip(y0 + 2.0, 0.0, 130.0)
    half = np.floor(pyA * 0.5)
    par = pyA - 2.0 * half
    px = np.clip(x0, -2.0, 128.0) + 2.0
    slot = par * NSLOT_E + half * WP2 + px
    # idx column block carries int32 BITS (bitcast-viewed as f32) so the
    # device needs no cast; weights follow as real f32
    idx_bits = slot.astype(np.int32).view(np.float32)
    meta = np.concatenate([idx_bits, wts.astype(np.float32)], axis=1)
    return np.ascontiguousarray(meta, dtype=np.float32)


def _in_maps(features, boxes):
    fgat = _prep_fgat(features)
    meta = _prep_wts_idx(boxes)
    return [
        {
            "fgat": fgat,
            "meta": np.ascontiguousarray(meta[k * P:(k + 1) * P]),
        }
        for k in range(NCORES)
    ]


def kernel(features, boxes, image_height=128, image_width=128):
    from concourse.bass_utils import run_bass_kernel_spmd

    if "nc" not in _STATE:
        _STATE["nc"] = _build_nc()
    nc = _STATE["nc"]

    in_maps = _in_maps(features, boxes)
    res = run_bass_kernel_spmd(
        nc, in_maps, core_ids=list(range(NCORES)),
        trace=_STATE.get("trace", False),
    )
    _STATE["last"] = res
    out = np.concatenate(
        [res.results[k]["out"].reshape(P, 7, 7, C).astype(np.float32)
         for k in range(NCORES)],
        axis=0,
    )
    return out
